# revision 1
# baseline (speedup 1.0000x reference)
"""Multi-head attention (B=2, S=2048, H=32, D=128) on 8 Trainium2 NeuronCores.

Sharding: tensor-parallel across heads.
  - core c owns heads [4c, 4c+4) (= feature slice [512c, 512(c+1)) of the
    projected dim) for BOTH batches.
  - Each core projects q/k/v (weights column-sharded by head) for all
    4096 tokens, runs attention for its 4 heads (batches never mix), then
    an 8-core AllToAll (split 2/1/1 over heads so transfers overlap the
    attention tail and the output projection start) reshards the context
    from head-major to token-major (512 tokens per core, global token
    index = b*2048 + s). Each core runs the full output projection on its
    token block, producing o^T [4096, 512]; host transposes + concatenates.

Layouts (per core, all device compute in bf16 with fp32 PSUM accumulation):
  - inputs arrive pre-transposed feature-major: qT/kT/vT [4096, 4096]
  - qpT/kpT [128, 4 heads, 4096 tokens] feature-major
  - vp [128, 32 token-tiles, 512 features] token-major
  - scores computed k-major S^T[k, q] in 2-bank PSUM tiles so each Exp
    activation covers 1024 columns (amortizes ACT instruction overhead);
    exp(P^T) feeds the ctx matmul directly as rhs (no transposes anywhere)
  - softmax row sums: DVE adds the 16 P^T tiles, then one ones[128,128]
    matmul fuses the partition-sum with the broadcast of s across
    partitions; 1/s (reciprocal_approx_fast) applied during ctx evacuation.
"""

import numpy as np
import ml_dtypes

import concourse.bacc as bacc
import concourse.mybir as mybir
import concourse.tile as tile
from concourse.bass_utils import run_bass_kernel_spmd

bf16 = ml_dtypes.bfloat16

B, S, H, D = 2, 2048, 32, 128
DM = H * D                      # 4096
BT = B * S                      # 4096 tokens total
N_CORES = 8
HL = H // N_CORES               # heads per core = 4
FL = HL * D                     # feature slice per core = 512
TB = BT // N_CORES              # output token block per core = 512
SCALE = float(D) ** -0.5

F32 = mybir.dt.float32
BF16 = mybir.dt.bfloat16

_CACHE = {}


def _build():
    nc = bacc.Bacc("TRN2", target_bir_lowering=False, debug=False,
                   num_devices=N_CORES)

    qT = nc.dram_tensor("qT", [DM, BT], BF16, kind="ExternalInput")
    kT = nc.dram_tensor("kT", [DM, BT], BF16, kind="ExternalInput")
    vT = nc.dram_tensor("vT", [DM, BT], BF16, kind="ExternalInput")
    wqT = nc.dram_tensor("wqT", [DM, FL], BF16, kind="ExternalInput")
    wkT = nc.dram_tensor("wkT", [DM, FL], BF16, kind="ExternalInput")
    wvT = nc.dram_tensor("wvT", [DM, FL], BF16, kind="ExternalInput")
    woT = nc.dram_tensor("woT", [DM, DM], BF16, kind="ExternalInput")
    outT = nc.dram_tensor("outT", [DM, TB], F32, kind="ExternalOutput")

    NKT = S // 128              # 16 k token-tiles per sequence
    Exp = mybir.ActivationFunctionType.Exp
    # AllToAll split over local heads: {0,1}, {2}, {3}
    A2A_H = [(0, 2), (2, 3), (3, 4)]

    with tile.TileContext(nc) as tc:
        with (
            tc.tile_pool(name="persist", bufs=1) as persist,
            tc.tile_pool(name="big", bufs=3, space="PSUM") as bigp,
            tc.tile_pool(name="ps", bufs=2, space="PSUM") as psp,
            tc.tile_pool(name="dram", bufs=1, space="DRAM") as dram,
        ):
            qpT = persist.tile([128, HL, BT], BF16, tag="qpT")
            kpT = persist.tile([128, HL, BT], BF16, tag="kpT")
            vp = persist.tile([128, BT // 128, FL], BF16, tag="vp")

            ones_m = persist.tile([128, 128], BF16, tag="ones_m")
            nc.vector.memset(ones_m[:], 1.0)

            # ---------------- phase 1: projections ----------------
            with tc.tile_pool(name="proj", bufs=2) as proj:
                for x_dram, w_dram, feat_major, out_t in (
                    (qT, wqT, True, qpT),
                    (kT, wkT, True, kpT),
                    (vT, wvT, False, vp),
                ):
                    x_r = x_dram.ap().rearrange("(kk p) t -> p kk t", p=128)
                    w_r = w_dram.ap().rearrange("(kk p) f -> p kk f", p=128)
                    wh = proj.tile([128, 32, FL], BF16, tag="w")
                    # split the weight load so the first matmuls start sooner
                    nc.sync.dma_start(out=wh[:, 0:16, :],
                                      in_=w_r[:, 0:16, :])
                    nc.sync.dma_start(out=wh[:, 16:32, :],
                                      in_=w_r[:, 16:32, :])
                    for tch in range(8):       # 512-token chunks
                        pss = [bigp.tile([128, 1024], F32, tag="big",
                                         name=f"pss{mp}")
                               for mp in range(2)]
                        for kh in range(2):    # halves of the contraction
                            xs = proj.tile([128, 16, 512], BF16, tag="xs")
                            nc.sync.dma_start(
                                out=xs[:],
                                in_=x_r[:, kh * 16:(kh + 1) * 16,
                                        tch * 512:(tch + 1) * 512])
                            for ms in range(4):
                                dst = pss[ms // 2][:, (ms % 2) * 512:
                                                   (ms % 2 + 1) * 512]
                                for kk in range(16):
                                    first = (kh == 0 and kk == 0)
                                    last = (kh == 1 and kk == 15)
                                    if feat_major:
                                        # out[f_out, t] += W^T.T @ xT
                                        nc.tensor.matmul(
                                            dst,
                                            wh[:, kh * 16 + kk,
                                               ms * 128:(ms + 1) * 128],
                                            xs[:, kk, :],
                                            start=first, stop=last)
                                    else:
                                        # out[t, f_out] += xT.T @ W^T
                                        nc.tensor.matmul(
                                            dst,
                                            xs[:, kk,
                                               ms * 128:(ms + 1) * 128],
                                            wh[:, kh * 16 + kk, :],
                                            start=first, stop=last)
                        for mp in range(2):
                            if feat_major:
                                dst = out_t[:, 2 * mp:2 * mp + 2,
                                            tch * 512:(tch + 1) * 512]
                            else:
                                dst = out_t[:, tch * 4 + 2 * mp:
                                            tch * 4 + 2 * mp + 2, :]
                            nc.vector.tensor_copy(dst, pss[mp][:])

            # ---------------- phase 2: attention ----------------
            in_bufs, out_bufs = [], []
            for gi, (h0, h1) in enumerate(A2A_H):
                in_bufs.append(dram.tile([N_CORES, h1 - h0, 128, TB], BF16,
                                         name=f"a2a_in{gi}"))
                out_bufs.append(dram.tile([N_CORES, h1 - h0, 128, TB], BF16,
                                          name=f"a2a_out{gi}"))

            with tc.tile_pool(name="attn", bufs=2) as attn:
                for hl in range(HL):
                    for b in range(B):
                        for qb in range(4):    # 512-query blocks
                            qs = slice(b * S + qb * TB, b * S + (qb + 1) * TB)
                            pt = attn.tile([128, NKT, TB], BF16, tag="pt",
                                           bufs=3)
                            for g in range(NKT // 2):   # k-tile pairs
                                st = bigp.tile([128, 1024], F32, tag="big")
                                for half in range(2):
                                    kt = 2 * g + half
                                    # S^T[k_tok, q] = khT.T @ qhT
                                    nc.tensor.matmul(
                                        st[:, half * 512:(half + 1) * 512],
                                        kpT[:, hl, b * S + kt * 128:
                                            b * S + (kt + 1) * 128],
                                        qpT[:, hl, qs],
                                        start=True, stop=True)
                                nc.scalar.activation(
                                    pt[:, 2 * g:2 * g + 2, :], st[:],
                                    Exp, scale=SCALE)
                            # partial row sums over the 16 k-tiles (DVE)
                            sp2 = attn.tile([128, 2, TB], BF16, tag="sp2",
                                            bufs=3)
                            nc.vector.tensor_add(sp2[:], pt[:, 0:2, :],
                                                 pt[:, 2:4, :])
                            for g in range(2, NKT // 2):
                                nc.vector.tensor_add(sp2[:], sp2[:],
                                                     pt[:, 2 * g:2 * g + 2, :])
                            sp = attn.tile([128, TB], BF16, tag="sp",
                                           bufs=3)
                            nc.vector.tensor_add(sp[:], sp2[:, 0, :],
                                                 sp2[:, 1, :])
                            # fused partition-sum + broadcast: ones.T @ sp
                            ps_b = psp.tile([128, TB], F32, tag="mm")
                            nc.tensor.matmul(ps_b[:], ones_m[:], sp[:],
                                             start=True, stop=True)
                            rsb = attn.tile([128, TB], F32, tag="rsb",
                                            bufs=3)
                            nc.vector.reciprocal_approx_fast(rsb[:], ps_b[:])
                            # ctx^T[d, q] = sum_kt vh[kt].T @ P^T[kt]
                            ps_c = psp.tile([128, TB], F32, tag="mm")
                            for kt in range(NKT):
                                nc.tensor.matmul(
                                    ps_c[:],
                                    vp[:, b * NKT + kt,
                                       hl * 128:(hl + 1) * 128],
                                    pt[:, kt, :],
                                    start=(kt == 0), stop=(kt == NKT - 1))
                            ctxs = attn.tile([128, TB], BF16, tag="ctxs",
                                             bufs=3)
                            nc.vector.tensor_tensor(
                                ctxs[:], ps_c[:], rsb[:],
                                op=mybir.AluOpType.mult)
                            for gi, (h0, h1) in enumerate(A2A_H):
                                if h0 <= hl < h1:
                                    nc.sync.dma_start(
                                        out=in_bufs[gi][b * 4 + qb, hl - h0],
                                        in_=ctxs[:])
                    for gi, (h0, h1) in enumerate(A2A_H):
                        if hl == h1 - 1:
                            nc.gpsimd.collective_compute(
                                "AllToAll", mybir.AluOpType.bypass,
                                replica_groups=[list(range(N_CORES))],
                                ins=[in_bufs[gi].opt()],
                                outs=[out_bufs[gi].opt()])

            # ---------------- phase 3: output projection ----------------
            # o^T[f_out, t] = sum over the 32 ctx feature tiles
            # global feature tile kk = j*HL + hl  (j = source rank)
            with tc.tile_pool(name="oproj", bufs=2) as op:
                ctxg = []
                for gi, (h0, h1) in enumerate(A2A_H):
                    cg = op.tile([128, N_CORES * (h1 - h0), TB], BF16,
                                 tag=f"ctxg{gi}", name=f"ctxg{gi}")
                    nc.sync.dma_start(
                        out=cg[:],
                        in_=out_bufs[gi].rearrange("j h p t -> p (j h) t"))
                    ctxg.append(cg)
                # accumulation order: all group-0 tiles, then 1, then 2 so
                # early matmuls run while later AllToAlls are in flight
                mm_seq = []
                for gi, (h0, h1) in enumerate(A2A_H):
                    for j in range(N_CORES):
                        for hl in range(h0, h1):
                            kk = j * HL + hl
                            mm_seq.append(
                                (kk, ctxg[gi], j * (h1 - h0) + hl - h0))

                wo_r = woT.ap().rearrange("(kk p) f -> p kk f", p=128)
                for fop in range(16):          # 256-wide out-feature pairs
                    woc = op.tile([128, 32, 256], BF16, tag="woc")
                    nc.sync.dma_start(
                        out=woc[:], in_=wo_r[:, :, fop * 256:(fop + 1) * 256])
                    ps_o = bigp.tile([128, 1024], F32, tag="big")
                    for sub in range(2):
                        dst = ps_o[:, sub * 512:(sub + 1) * 512]
                        for n_mm, (kk, cg, ci) in enumerate(mm_seq):
                            nc.tensor.matmul(
                                dst,
                                woc[:, kk, sub * 128:(sub + 1) * 128],
                                cg[:, ci, :],
                                start=(n_mm == 0), stop=(n_mm == 31))
                    ot = op.tile([128, 2, TB], F32, tag="ot")
                    nc.vector.tensor_copy(ot[:], ps_o[:])
                    nc.sync.dma_start(
                        out=outT.ap().rearrange(
                            "(fo p) t -> p fo t", p=128)[
                            :, fop * 2:fop * 2 + 2, :],
                        in_=ot[:])

    nc.compile()
    return nc


def _prep_inputs(q, k, v, Wq, Wk, Wv, Wo):
    """Host-side sharding: cast to bf16, transpose to feature-major, slice."""
    q, k, v = (np.asarray(x, dtype=np.float32) for x in (q, k, v))
    Wq, Wk, Wv, Wo = (np.asarray(x, dtype=np.float32)
                      for x in (Wq, Wk, Wv, Wo))
    qT = np.ascontiguousarray(q.reshape(BT, DM).astype(bf16).T)
    kT = np.ascontiguousarray(k.reshape(BT, DM).astype(bf16).T)
    vT = np.ascontiguousarray(v.reshape(BT, DM).astype(bf16).T)
    woT = np.ascontiguousarray(Wo.astype(bf16).T)
    in_maps = []
    for c in range(N_CORES):
        sl = slice(c * FL, (c + 1) * FL)
        in_maps.append({
            "qT": qT, "kT": kT, "vT": vT,
            "wqT": np.ascontiguousarray(Wq[sl, :].astype(bf16).T),
            "wkT": np.ascontiguousarray(Wk[sl, :].astype(bf16).T),
            "wvT": np.ascontiguousarray(Wv[sl, :].astype(bf16).T),
            "woT": woT,
        })
    return in_maps


def run_spmd(inputs, trace=False):
    if "nc" not in _CACHE:
        _CACHE["nc"] = _build()
    nc = _CACHE["nc"]
    in_maps = _prep_inputs(**inputs)
    res = run_bass_kernel_spmd(nc, in_maps, core_ids=list(range(N_CORES)),
                               trace=trace)
    o = np.empty((BT, DM), dtype=np.float32)
    for c in range(N_CORES):
        o[c * TB:(c + 1) * TB, :] = res.results[c]["outT"].T
    return o.reshape(B, S, DM), res


def kernel(q, k, v, Wq, Wk, Wv, Wo):
    o, _ = run_spmd(dict(q=q, k=k, v=v, Wq=Wq, Wk=Wk, Wv=Wv, Wo=Wo))
    return o



# revision 11
# speedup vs baseline: 1.2647x; 1.2647x over previous
"""Multi-head attention (B=2, S=2048, H=32, D=128) on 8 Trainium2 NeuronCores.

Sharding: tensor-parallel across heads (core c owns heads [4c, 4c+4)).
Each core projects q/k/v for all 4096 tokens (weights column-sharded by
head), runs attention for its 4 heads, reshards the context head-major ->
token-major with one AllToAll per head, and runs the full output projection
on its 512-token block, producing o^T [4096, 512] (host transposes).

Scheduling: the tensor engine is the bottleneck (~260 ns per 512-free
matmul regardless of dtype/shape), so the kernel is one continuous PE
stream with the stalls scheduled away:
  P1 k-proj | P2 q-proj | P3 v-proj(b0)  -- dense GEMMs, ScalarE evacuates
  P4 attention(b0): blocks software-pipelined (ctx of block n-1 and its
     softmax tail interleaved into the scores of block n so the PE never
     waits on the Exp activations); v-proj(b1) chains fill remaining gaps
  P5 attention(b1): AllToAll fires per head as it completes; o-proj
     partial-chain bursts for landed groups keep the PE fed
  P6 last two o-proj groups (the second-to-last burst hides the final
     AllToAll) + bf16-accumulator combine + output DMA
PSUM: 2x[128,1024] (scores / kq-proj) + 4x[128,512] (v-proj, ctx, row-sum,
o-proj chains) = exactly 8 banks.
"""

import numpy as np
import ml_dtypes

import concourse.bacc as bacc
import concourse.mybir as mybir
import concourse.tile as tile
from concourse.bass_utils import run_bass_kernel_spmd

bf16 = ml_dtypes.bfloat16

B, S, H, D = 2, 2048, 32, 128
DM = H * D                      # 4096
BT = B * S                      # 4096 tokens total
N_CORES = 8
HL = H // N_CORES               # heads per core = 4
FL = HL * D                     # feature slice per core = 512
TB = BT // N_CORES              # output token block per core = 512
NKT = S // 128                  # 16 k token-tiles per sequence
SCALE = float(D) ** -0.5

F32 = mybir.dt.float32
BF16 = mybir.dt.bfloat16
Exp = mybir.ActivationFunctionType.Exp
Copy = mybir.ActivationFunctionType.Copy

_CACHE = {}


def _build():
    nc = bacc.Bacc("TRN2", target_bir_lowering=False, debug=False,
                   num_devices=N_CORES)

    qT = nc.dram_tensor("qT", [DM, BT], BF16, kind="ExternalInput")
    kT = nc.dram_tensor("kT", [DM, BT], BF16, kind="ExternalInput")
    vT = nc.dram_tensor("vT", [DM, BT], BF16, kind="ExternalInput")
    wqT = nc.dram_tensor("wqT", [DM, FL], BF16, kind="ExternalInput")
    wkT = nc.dram_tensor("wkT", [DM, FL], BF16, kind="ExternalInput")
    wvT = nc.dram_tensor("wvT", [DM, FL], BF16, kind="ExternalInput")
    woT = nc.dram_tensor("woT", [DM, DM], BF16, kind="ExternalInput")
    outT = nc.dram_tensor("outT", [DM, TB], F32, kind="ExternalOutput")

    v_r = vT.ap().rearrange("(kk p) t -> p kk t", p=128)
    wo_r = woT.ap().rearrange("(kk p) f -> p kk f", p=128)
    out_r = outT.ap().rearrange("(fo p) t -> p fo t", p=128)

    with tile.TileContext(nc) as tc:
        with (
            tc.tile_pool(name="persist", bufs=1) as persist,
            tc.tile_pool(name="big", bufs=2, space="PSUM") as bigp,
            tc.tile_pool(name="sm", bufs=4, space="PSUM") as smp,
            tc.tile_pool(name="attn", bufs=1) as attn,
            tc.tile_pool(name="dram", bufs=1, space="DRAM") as dram,
        ):
            qpT = persist.tile([128, HL, BT], BF16, tag="qpT")
            kpT = persist.tile([128, HL, BT], BF16, tag="kpT")
            vp = persist.tile([128, B * NKT, FL], BF16, tag="vp")
            ones_m = persist.tile([128, 128], BF16, tag="ones_m")
            nc.vector.memset(ones_m[:], 1.0)

            in_bufs, out_bufs = [], []
            for h in range(HL):
                in_bufs.append(dram.tile([N_CORES, 128, TB], BF16,
                                         tag=f"ain{h}", name=f"a2a_in{h}"))
                out_bufs.append(dram.tile([N_CORES, 128, TB], BF16,
                                          tag=f"aout{h}", name=f"a2a_out{h}"))

            # ---------------- P1/P2: k then q projection (feature-major) ---
            with (
                tc.tile_pool(name="wkq", bufs=2) as wkq,
                tc.tile_pool(name="xkq", bufs=2) as xkq,
            ):
                for x_dram, w_dram, out_t in ((kT, wkT, kpT), (qT, wqT, qpT)):
                    x_r = x_dram.ap().rearrange("(kk p) t -> p kk t", p=128)
                    w_r = w_dram.ap().rearrange("(kk p) f -> p kk f", p=128)
                    wh = []
                    for kh in range(2):
                        w = wkq.tile([128, 16, FL], BF16, tag="w", name="w")
                        nc.sync.dma_start(
                            out=w[:], in_=w_r[:, kh * 16:(kh + 1) * 16, :])
                        wh.append(w)
                    for tch in range(8):       # 512-token chunks
                        pss = [bigp.tile([128, 1024], F32, tag="big",
                                         name=f"pss{mp}") for mp in range(2)]
                        for kh in range(2):    # halves of the contraction
                            xs = xkq.tile([128, 16, 512], BF16, tag="xs")
                            nc.sync.dma_start(
                                out=xs[:],
                                in_=x_r[:, kh * 16:(kh + 1) * 16,
                                        tch * 512:(tch + 1) * 512])
                            for ms in range(4):
                                dst = pss[ms // 2][:, (ms % 2) * 512:
                                                   (ms % 2 + 1) * 512]
                                for kk in range(16):
                                    nc.tensor.matmul(
                                        dst,
                                        wh[kh][:, kk,
                                               ms * 128:(ms + 1) * 128],
                                        xs[:, kk, :],
                                        start=(kh == 0 and kk == 0),
                                        stop=(kh == 1 and kk == 15))
                        for mp in range(2):
                            nc.scalar.activation(
                                out_t[:, 2 * mp:2 * mp + 2,
                                      tch * 512:(tch + 1) * 512],
                                pss[mp][:], Copy)

            # ---------------- fill queue (gap-filler steps for the PE) -----
            fill_q = []

            def fill(budget):
                while fill_q and budget > 0:
                    cost, fn = fill_q.pop(0)
                    budget -= cost
                    fn()

            # -------- software-pipelined attention block machinery ---------
            pend = [None]

            def attn_iter(cur):
                """Emit scores+exp for block `cur`; weave in the softmax
                tail and the ctx matmuls of the previous block."""
                p = pend[0]

                def ctx_pair(kt0):
                    if p["ps_c"] is None:
                        p["ps_c"] = smp.tile([128, TB], F32, tag="sm",
                                             name="ps_c")
                    for kt in (kt0, kt0 + 1):
                        nc.tensor.matmul(
                            p["ps_c"][:],
                            vp[:, p["b"] * NKT + kt,
                               p["hl"] * 128:(p["hl"] + 1) * 128],
                            p["pt"][:, kt, :],
                            start=(kt == 0), stop=(kt == NKT - 1))

                def finish_sums():
                    sp = attn.tile([128, TB], BF16, tag="sp", bufs=2)
                    nc.vector.tensor_add(sp[:], p["sp2"][:, 0, :],
                                         p["sp2"][:, 1, :])
                    ps_b = smp.tile([128, TB], F32, tag="sm", name="ps_b")
                    nc.tensor.matmul(ps_b[:], ones_m[:], sp[:],
                                     start=True, stop=True)
                    rsb = attn.tile([128, TB], F32, tag="rsb", bufs=2)
                    nc.vector.reciprocal_approx_fast(rsb[:], ps_b[:])
                    p["rsb"] = rsb

                def finish_ctx():
                    ctxs = attn.tile([128, TB], BF16, tag="ctxs", bufs=2)
                    nc.vector.tensor_tensor(ctxs[:], p["ps_c"][:],
                                            p["rsb"][:],
                                            op=mybir.AluOpType.mult)
                    nc.sync.dma_start(
                        out=in_bufs[p["hl"]][p["b"] * 4 + p["qb"]],
                        in_=ctxs[:])

                if cur is None:            # final flush
                    if p is not None:
                        finish_sums()
                        for g in range(8):
                            ctx_pair(2 * g)
                        finish_ctx()
                        pend[0] = None
                    return

                hl, b, qb = cur
                qs = slice(b * S + qb * TB, b * S + (qb + 1) * TB)
                pt = attn.tile([128, NKT, TB], BF16, tag="pt", bufs=2)
                sp2 = attn.tile([128, 2, TB], BF16, tag="sp2", bufs=2)
                for g in range(8):
                    st = bigp.tile([128, 1024], F32, tag="big")
                    for half in range(2):
                        kt = 2 * g + half
                        nc.tensor.matmul(
                            st[:, half * 512:(half + 1) * 512],
                            kpT[:, hl, b * S + kt * 128:
                                b * S + (kt + 1) * 128],
                            qpT[:, hl, qs],
                            start=True, stop=True)
                    nc.scalar.activation(pt[:, 2 * g:2 * g + 2, :],
                                         st[:], Exp, scale=SCALE)
                    if p is not None and g >= 1:
                        ctx_pair(2 * (g - 1))
                    if g == 7 and p is not None:
                        ctx_pair(14)
                        finish_ctx()       # before add7 so DVE isn't blocked
                    if g == 1:
                        nc.vector.tensor_add(sp2[:], pt[:, 0:2, :],
                                             pt[:, 2:4, :])
                        if p is not None:
                            finish_sums()
                        fill(1)
                    elif g > 1:
                        nc.vector.tensor_add(sp2[:], sp2[:],
                                             pt[:, 2 * g:2 * g + 2, :])
                        if g == 5:
                            fill(1)
                pend[0] = {"hl": hl, "b": b, "qb": qb, "pt": pt, "sp2": sp2,
                           "rsb": None, "ps_c": None}

            # ---------------- P3 + P4 (v-proj scoped) ----------------------
            with (
                tc.tile_pool(name="wvp", bufs=1) as wvp,
                tc.tile_pool(name="xvp", bufs=2) as xvp,
            ):
                wv = wvp.tile([128, 32, FL], BF16, tag="wv")
                wv_r = wvT.ap().rearrange("(kk p) f -> p kk f", p=128)
                nc.sync.dma_start(out=wv[:, 0:16, :], in_=wv_r[:, 0:16, :])
                nc.sync.dma_start(out=wv[:, 16:32, :], in_=wv_r[:, 16:32, :])

                def vproj_chunk_steps(c, evac_scalar):
                    """(cost, fn) steps for one 512-token chunk of v-proj.
                    Each k-tile chain (32 matmuls into one PSUM bank) is
                    split into 4 parts of 8 matmuls for even spreading."""
                    steps = []
                    xh = [None, None]
                    psh = [None]

                    def dma(kh, c=c):
                        xh[kh] = xvp.tile([128, 16, 512], BF16, tag="xs",
                                          name="xs")
                        nc.sync.dma_start(
                            out=xh[kh][:],
                            in_=v_r[:, kh * 16:(kh + 1) * 16,
                                    c * 512:(c + 1) * 512])
                    steps.append((0, lambda: dma(0)))
                    steps.append((0, lambda: dma(1)))
                    for kt4 in range(4):
                        for part in range(4):
                            def pstep(kt4=kt4, part=part, c=c):
                                kh, k8 = part // 2, (part % 2) * 8
                                if part == 0:
                                    psh[0] = smp.tile([128, FL], F32,
                                                      tag="sm", name="ps_v")
                                ps = psh[0]
                                for kk in range(k8, k8 + 8):
                                    nc.tensor.matmul(
                                        ps[:],
                                        xh[kh][:, kk,
                                               kt4 * 128:(kt4 + 1) * 128],
                                        wv[:, kh * 16 + kk, :],
                                        start=(part == 0 and kk == k8),
                                        stop=(part == 3 and kk == k8 + 7))
                                if part == 3:
                                    kt = c * 4 + kt4
                                    if evac_scalar:
                                        nc.scalar.activation(vp[:, kt, :],
                                                             ps[:], Copy)
                                    else:
                                        nc.vector.tensor_copy(vp[:, kt, :],
                                                              ps[:])
                            steps.append((8, pstep))
                    return steps

                # P3: v-proj for batch 0, dense.
                for c in range(4):
                    for _, fn in vproj_chunk_steps(c, evac_scalar=True):
                        fn()

                # P4: attention b0 with v-proj b1 chains as gap fillers.
                for hl in range(HL):
                    fill_q.extend(
                        vproj_chunk_steps(4 + hl, evac_scalar=False))
                    for qb in range(4):
                        attn_iter((hl, 0, qb))
                    fill(10 ** 9)   # drain at head boundary

            # ---------------- P5 + P6 (o-proj scoped) ----------------------
            with tc.tile_pool(name="oproj", bufs=1) as op:
                acc = op.tile([128, 16, 2, TB], BF16, tag="acc")
                ctxg = [None] * HL

                def load_ctxg(h):
                    # shared 2-deep ring: group h reuses the buffer of group
                    # h-2, which the burst schedule has already consumed
                    ctxg[h] = op.tile([128, N_CORES, TB], BF16,
                                      tag="ctxg", bufs=2, name=f"ctxg{h}")
                    nc.sync.dma_start(
                        out=ctxg[h][:],
                        in_=out_bufs[h].rearrange("j p t -> p j t"))

                def collective(h):
                    nc.gpsimd.collective_compute(
                        "AllToAll", mybir.AluOpType.bypass,
                        replica_groups=[list(range(N_CORES))],
                        ins=[in_bufs[h].opt()],
                        outs=[out_bufs[h].opt()])
                    load_ctxg(h)

                def oproj_group(h):
                    """Partial chains for one A2A group: 16 fop x 2 sub.
                    woT rows are host-permuted so group h's 8 contraction
                    tiles are rows [h*1024, (h+1)*1024)."""
                    for fop in range(16):
                        woc = op.tile([128, N_CORES, 256], BF16,
                                      tag="woc", bufs=2, name="woc")
                        nc.sync.dma_start(
                            out=woc[:],
                            in_=wo_r[:, h * N_CORES:(h + 1) * N_CORES,
                                     fop * 256:(fop + 1) * 256])
                        for sub in range(2):
                            ps = smp.tile([128, TB], F32, tag="sm",
                                          name="ps_o")
                            for j in range(N_CORES):
                                nc.tensor.matmul(
                                    ps[:],
                                    woc[:, j, sub * 128:(sub + 1) * 128],
                                    ctxg[h][:, j, :],
                                    start=(j == 0), stop=(j == N_CORES - 1))
                            a = acc[:, fop, sub, :]
                            if h == 0:
                                nc.vector.tensor_copy(a, ps[:])
                            elif h < HL - 1:
                                nc.vector.tensor_add(a, a, ps[:])
                            else:
                                ot = op.tile([128, TB], F32, tag="ot",
                                             bufs=2, name="ot")
                                nc.vector.tensor_add(ot[:], a, ps[:])
                                nc.sync.dma_start(
                                    out=out_r[:, fop * 2 + sub, :],
                                    in_=ot[:])

                # P5: attention b1.  Block (hl,1,3) finishes during the next
                # iteration, so collective(hl) is emitted one block later;
                # o-proj bursts trail each A2A by >= 5 attention blocks.
                for i in range(16):
                    hl, qb = i // 4, i % 4
                    attn_iter((hl, 1, qb))
                    if i in (4, 8, 12):
                        collective(i // 4 - 1)
                    if i == 10:
                        oproj_group(0)
                    if i == 14:
                        oproj_group(1)

                # P6: flush the last block, fire the last A2A, then the
                # group-2 burst hides it before group 3 runs.
                attn_iter(None)
                collective(3)
                oproj_group(2)
                oproj_group(3)

    nc.compile()
    return nc


def _prep_inputs(q, k, v, Wq, Wk, Wv, Wo):
    """Host-side sharding: cast to bf16, transpose to feature-major, slice."""
    q, k, v = (np.asarray(x, dtype=np.float32) for x in (q, k, v))
    Wq, Wk, Wv, Wo = (np.asarray(x, dtype=np.float32)
                      for x in (Wq, Wk, Wv, Wo))
    qT = np.ascontiguousarray(q.reshape(BT, DM).astype(bf16).T)
    kT = np.ascontiguousarray(k.reshape(BT, DM).astype(bf16).T)
    vT = np.ascontiguousarray(v.reshape(BT, DM).astype(bf16).T)
    # woT rows permuted so contraction tile kk' = h*8 + j holds the global
    # feature tile kk = j*HL + h (group-contiguous for the kernel).
    woT = Wo.astype(bf16).T.reshape(N_CORES, HL, 128, DM)
    woT = np.ascontiguousarray(woT.transpose(1, 0, 2, 3).reshape(DM, DM))
    in_maps = []
    for c in range(N_CORES):
        sl = slice(c * FL, (c + 1) * FL)
        in_maps.append({
            "qT": qT, "kT": kT, "vT": vT,
            "wqT": np.ascontiguousarray(Wq[sl, :].astype(bf16).T),
            "wkT": np.ascontiguousarray(Wk[sl, :].astype(bf16).T),
            "wvT": np.ascontiguousarray(Wv[sl, :].astype(bf16).T),
            "woT": woT,
        })
    return in_maps


def run_spmd(inputs, trace=False):
    if "nc" not in _CACHE:
        _CACHE["nc"] = _build()
    nc = _CACHE["nc"]
    in_maps = _prep_inputs(**inputs)
    res = run_bass_kernel_spmd(nc, in_maps, core_ids=list(range(N_CORES)),
                               trace=trace)
    o = np.empty((BT, DM), dtype=np.float32)
    for c in range(N_CORES):
        o[c * TB:(c + 1) * TB, :] = res.results[c]["outT"].T
    return o.reshape(B, S, DM), res


def kernel(q, k, v, Wq, Wk, Wv, Wo):
    o, _ = run_spmd(dict(q=q, k=k, v=v, Wq=Wq, Wk=Wk, Wv=Wv, Wo=Wo))
    return o
